# revision 15
# baseline (speedup 1.0000x reference)
"""Causal self-attention Trainium2 kernel (8 NeuronCores).

Sharding: core = b*2 + g where b = batch (4), g = head-group (2 groups x 8 heads).
Each core computes, for its (batch, head-group):
    qkv = x[b] @ w_attn[:, group cols] + b_attn[group]
    y_g = softmax_causal(q k^T / sqrt(hs)) v          (8 heads)
    part = y_g @ w_proj[group rows, :] (+ b_proj on g==0)
Host sums the two per-batch partials (the c_proj row-split reduction).

v3 design (vs v2):
  - x is transposed + cast to bf16 on the host: xT DMAs straight into its
    [128, CK, T] SBUF layout. Kills the PE-transpose pipeline (128 transposes,
    psum->sbuf copies, f32->bf16 casts) and halves the x DMA bytes.
  - Attention runs per head PAIR: the two heads of a pair live on partition
    halves 0-63 / 64-127 of qkT, so their K=64 QK^T matmuls map to PE row
    tiles (0,0) and (64,0) and stream concurrently (2x QK throughput).
  - Causal diagonal blocks are range-restricted: QK^T per-tile valid-q lower
    bound, exp/mask/AV at pair granularity. Skips ~60% of the wasted
    masked-region work on PE and ACT.
  - w_qkv is re-tiled on host per 128-col m-block so the first QK matmul only
    waits on a 256KB DMA; xT DMAs are per 512-token block.
  - exp on scalar (ACT); bias adds / mask / normalize on DVE; y partials
    stored bf16 (host accumulates in f32).
"""

import sys

sys.path.insert(0, "/opt/trn_rl_repo")

import math
import numpy as np
import ml_dtypes

import concourse.bass as bass
import concourse.bacc as bacc
import concourse.tile as tile
from concourse import mybir
from concourse import bass_utils


def _ensure_ntff_hook():
    """Provide antenv.axon_hooks (NTFF profiling registry) if the image's
    antenv lacks it, wiring the ctypes-based hook from trn_agent_boot."""
    import types
    try:
        import antenv.axon_hooks  # noqa: F401
        return
    except ImportError:
        pass
    try:
        import antenv
        from trn_agent_boot.trn_boot import _ntff_profile_via_ctypes
        hook = _ntff_profile_via_ctypes("/opt/axon/libaxon_pjrt.so")
    except Exception:
        return
    mod = types.ModuleType("antenv.axon_hooks")
    mod.get_axon_ntff_profile_hook = lambda: hook
    mod.set_axon_ntff_profile_hook = lambda h: None
    sys.modules["antenv.axon_hooks"] = mod
    antenv.axon_hooks = mod


_ensure_ntff_hook()

F32 = mybir.dt.float32
BF16 = mybir.dt.bfloat16
AF = mybir.ActivationFunctionType
ALU = mybir.AluOpType

T = 2048
C = 1024
HS = 64           # head size
NHL = 8           # heads per core
GC = NHL * HS     # 512: group width
CK = C // 128     # 8 contraction tiles for qkv
MT = T // 128     # 16 row tiles
QB = 512          # q block (one fp32 PSUM bank)
NQ = T // QB      # 4
NM = 3 * GC // 128  # 12 m-blocks of w_qkv
SCALE = 1.0 / math.sqrt(HS)
N_CORES = 8


def build_program():
    nc = bacc.Bacc("TRN2", target_bir_lowering=False, debug=False, num_devices=N_CORES)
    xT_d = nc.dram_tensor("xT", [C, T], BF16, kind="ExternalInput").ap()
    wqkv_d = nc.dram_tensor("w_qkv", [128, CK, NM * 128], BF16, kind="ExternalInput").ap()
    bqk_d = nc.dram_tensor("b_qk", [128, 8], F32, kind="ExternalInput").ap()
    bv_d = nc.dram_tensor("b_v", [GC], F32, kind="ExternalInput").ap()
    wproj_d = nc.dram_tensor("w_proj", [GC, C], BF16, kind="ExternalInput").ap()
    bproj_d = nc.dram_tensor("b_proj", [C], F32, kind="ExternalInput").ap()
    masks_d = nc.dram_tensor("masks", [4, 128, QB], BF16, kind="ExternalInput").ap()
    y_d = nc.dram_tensor("y", [T, C], BF16, kind="ExternalOutput").ap()

    def bcast(ap, parts):
        # replicate a [1, N] slice across `parts` partitions (DMA source AP)
        return bass.AP(tensor=ap.tensor, offset=ap.offset, ap=[[0, parts]] + list(ap.ap)[-1:])

    with tile.TileContext(nc) as tc:
        from contextlib import ExitStack

        with ExitStack() as ctx:
            const = ctx.enter_context(tc.tile_pool(name="const", bufs=1))
            sexp = ctx.enter_context(tc.tile_pool(name="sexp", bufs=6))
            csp = ctx.enter_context(tc.tile_pool(name="csp", bufs=4))
            nrm = ctx.enter_context(tc.tile_pool(name="nrm", bufs=4))
            ost = ctx.enter_context(tc.tile_pool(name="ost", bufs=3))
            pss = ctx.enter_context(tc.tile_pool(name="pss", bufs=2, space="PSUM"))
            pys = ctx.enter_context(tc.tile_pool(name="pys", bufs=2, space="PSUM"))
            pgen = ctx.enter_context(tc.tile_pool(name="pgen", bufs=2, space="PSUM"))

            # ---------------- persistent activations ----------------
            xT = const.tile([128, CK, T], BF16)
            qkT = const.tile([128, 8, T], BF16)   # m 0..3 = q cols, 4..7 = k cols
            v = const.tile([128, MT, NHL, HS + 1], BF16)
            yTu = const.tile([128, 4, T], BF16)   # unnormalized y^T (head-dim major)

            # ---------------- constants + input DMAs ----------------
            # The first QK matmul (block 0, m=0) needs w_qkv m-block 0 and xT
            # block 0 only; order the DMAs so those land first, split across
            # the sync (w) and gpsimd (xT) queues.
            w_qkv = const.tile([128, CK, NM * 128], BF16)
            ones = const.tile([128, HS], BF16)
            nc.vector.memset(ones, 1.0)
            nc.vector.memset(v[:, :, :, HS:HS + 1], 1.0)

            # DMA plan: the startup critical path is xT block 0 (gpsimd,
            # per-c so the first qkv chain can start after 128KB) and the
            # w_qkv m-blocks needed by the first QK matmuls.  The SBUF (and
            # host) column order of w_qkv is the NEEDED order, so the w DMAs
            # coalesce into 5 contiguous triggers on sync.  The scalar queue
            # carries no DMAs (a blocked trigger there delays the first exp).
            xT_r = xT_d.rearrange("(c p) t -> p c t", p=128)
            b_qk = const.tile([128, 8], F32)
            nc.gpsimd.dma_start(out=b_qk, in_=bqk_d)
            for c in range(CK):
                nc.gpsimd.dma_start(out=xT[:, c, 0:QB], in_=xT_r[:, c, 0:QB])
            for j0, j1 in [(0, 2), (2, 6), (6, 8), (8, 10), (10, 12)]:
                nc.sync.dma_start(out=w_qkv[:, :, j0 * 128:j1 * 128],
                                  in_=wqkv_d[:, :, j0 * 128:j1 * 128])
            b_v = const.tile([128, GC], F32)
            nc.gpsimd.dma_start(out=b_v, in_=bcast(bv_d, 128))
            for blk in range(1, NQ):
                nc.gpsimd.dma_start(out=xT[:, :, blk * QB:(blk + 1) * QB],
                                    in_=xT_r[:, :, blk * QB:(blk + 1) * QB])
            mask = const.tile([128, 4, QB], BF16)
            nc.gpsimd.dma_start(out=mask, in_=masks_d.rearrange("d p q -> p d q"))
            w_proj = const.tile([128, 4, C], BF16)
            nc.sync.dma_start(out=w_proj, in_=wproj_d.rearrange("(c p) n -> p c n", p=128))
            b_proj = const.tile([128, C], F32)
            nc.gpsimd.dma_start(out=b_proj, in_=bcast(bproj_d, 128))

            # SBUF/host column order of w_qkv (DMA-needed order)
            W_ORDER = [0, 4, 8, 9, 10, 11, 1, 5, 2, 6, 3, 7]
            WPOS = {m: i for i, m in enumerate(W_ORDER)}

            def make_qk_closure(n, m):
                """qk-gen matmul chain for m-block m of token block n."""
                cols = slice(n * QB, (n + 1) * QB)
                wp = WPOS[m]

                def QK():
                    ps = pgen.tile([128, QB], F32, tag="mm")
                    for c in range(CK):
                        nc.tensor.matmul(ps,
                                         lhsT=w_qkv[:, c, wp * 128:(wp + 1) * 128],
                                         rhs=xT[:, c, cols],
                                         start=(c == 0), stop=(c == CK - 1))
                    nc.vector.tensor_scalar_add(out=qkT[:, m, cols],
                                                in0=ps, scalar1=b_qk[:, m:m + 1])
                return QK

            def make_vmm_closure(t):
                """v-gen matmul chain for 128-token tile t."""
                def VMM():
                    ps = pgen.tile([128, QB], F32, tag="mm")
                    for c in range(CK):
                        nc.tensor.matmul(ps,
                                         lhsT=xT[:, c, t * 128:(t + 1) * 128],
                                         rhs=w_qkv[:, c, 2 * 128:6 * 128],
                                         start=(c == 0), stop=(c == CK - 1))
                    nc.vector.tensor_tensor(out=v[:, t, :, 0:HS],
                                            in0=ps.rearrange("p (h d) -> p h d", d=HS),
                                            in1=b_v.rearrange("p (h d) -> p h d", d=HS),
                                            op=ALU.add)
                return VMM

            def make_proj_closures(n):
                """projection of q-block n (requires normalized yTu block n)."""
                cls = []
                for t in range(4 * n, 4 * n + 4):
                    for n2 in range(2):
                        def P(t=t, n2=n2):
                            ps = pgen.tile([128, QB], F32, tag="mm")
                            for c4 in range(4):
                                nc.tensor.matmul(ps,
                                                 lhsT=yTu[:, c4, t * 128:(t + 1) * 128],
                                                 rhs=w_proj[:, c4, n2 * QB:(n2 + 1) * QB],
                                                 start=(c4 == 0), stop=(c4 == 3))
                            ot = ost.tile([128, QB], BF16, tag="ot")
                            nc.vector.tensor_tensor(out=ot, in0=ps,
                                                    in1=b_proj[:, n2 * QB:(n2 + 1) * QB],
                                                    op=ALU.add)
                            nc.sync.dma_start(
                                out=y_d[t * 128:(t + 1) * 128, n2 * QB:(n2 + 1) * QB],
                                in_=ot)
                        cls.append(P)
                return cls

            def emit_pair(qj, hp, carry, filler):
                """attention for head pair (2hp, 2hp+1) on q-block qj.

                Head A = 2hp lives on partitions 0-63, head B = 2hp+1 on
                64-127 of qkT column hp (q) / 4+hp (k): the pair's QK^T
                matmuls are emitted back-to-back so they run concurrently on
                PE row tiles (0,0) and (64,0).  AV(kp-1) trails QK(kp) by one
                step.  `carry` (the previous pair's finish closures) runs at
                step 0 so its PSUM readers precede this pair's first AV;
                `filler` (qkv of a later block / proj of an earlier one) is
                drained evenly across the remaining steps to keep the PE fed
                under the exp chain.
                """
                q0 = qj * QB
                nki = 4 * (qj + 1)
                exs = {}
                # previous pair's finish-copies first: their ACT/DVE ops front
                # the queues so the py PSUM slots this pair reuses free early
                for cl in carry:
                    cl()
                pyA = pys.tile([HS + 1, QB], F32, tag="py", name=f"pyA{qj}_{hp}")
                pyB = pys.tile([HS + 1, QB], F32, tag="py", name=f"pyB{qj}_{hp}")
                for step in range(nki + 1):
                    if step < nki:
                        ki = step
                        # valid-q lower bound (diagonal restriction)
                        lo = max(0, (ki - 4 * qj) * 128)
                        ps = pss.tile([128, 2, QB], F32, tag="s")
                        nc.tensor.matmul(ps[:, 0, lo:],
                                         lhsT=qkT[0:HS, 4 + hp, ki * 128:(ki + 1) * 128],
                                         rhs=qkT[0:HS, hp, q0 + lo:q0 + QB],
                                         start=True, stop=True)
                        nc.tensor.matmul(ps[:, 1, lo:],
                                         lhsT=qkT[HS:128, 4 + hp, ki * 128:(ki + 1) * 128],
                                         rhs=qkT[HS:128, hp, q0 + lo:q0 + QB],
                                         start=True, stop=True)
                        ex = sexp.tile([128, 2, QB], BF16, tag="e")
                        nc.scalar.activation(out=ex[:, :, lo:], in_=ps[:, :, lo:],
                                             func=AF.Exp, scale=SCALE)
                        dk = ki - 4 * qj
                        if dk >= 0:
                            for j in range(2):
                                nc.vector.tensor_tensor(out=ex[:, j, lo:],
                                                        in0=ex[:, j, lo:],
                                                        in1=mask[:, dk, lo:],
                                                        op=ALU.mult)
                        exs[ki] = (ex, lo)
                    if step > 0:
                        ki = step - 1
                        ex, lo = exs.pop(ki)
                        nc.tensor.matmul(pyA[:, lo:], lhsT=v[:, ki, 2 * hp, :],
                                         rhs=ex[:, 0, lo:],
                                         start=(ki == 0), stop=(ki == nki - 1))
                        nc.tensor.matmul(pyB[:, lo:], lhsT=v[:, ki, 2 * hp + 1, :],
                                         rhs=ex[:, 1, lo:],
                                         start=(ki == 0), stop=(ki == nki - 1))
                        steps_left = nki - (step - 1)
                        n_pop = (len(filler) + steps_left - 1) // steps_left
                        for _ in range(n_pop):
                            filler.pop(0)()
                return pyA, pyB

            def make_finish_closures(qj, hp, pyA, pyB):
                """denominator/y^T copies + normalize for head pair hp of
                q-block qj, split in two so the PE-side normalize lands a
                couple of steps after the copies it depends on."""
                cols = slice(qj * QB, (qj + 1) * QB)
                cst = csp.tile([128, QB], F32, tag="cs", name=f"cs{qj}_{hp}")

                def COPIES():
                    # denominator rows -> partitions 0 / 64 of one tile.
                    # gpsimd can't touch PSUM and DVE mishandles partition-
                    # shifted copies, so the two shifted ones go to ACT.
                    nc.scalar.copy(out=cst[0:1, :], in_=pyA[HS:HS + 1, :])
                    nc.vector.tensor_copy(out=cst[HS:HS + 1, :], in_=pyB[HS:HS + 1, :])
                    # y^T copies: A base-aligned on DVE, B shifted on ACT
                    nc.vector.tensor_copy(out=yTu[0:HS, hp, cols], in_=pyA[0:HS, :])
                    nc.scalar.copy(out=yTu[HS:128, hp, cols], in_=pyB[0:HS, :])

                def NORM():
                    rcp = nrm.tile([128, QB], F32, tag="rcp")
                    nc.vector.reciprocal_approx_fast(out=rcp, in_=cst)
                    r16 = nrm.tile([128, QB], BF16, tag="r16")
                    nc.vector.tensor_copy(out=r16, in_=rcp)
                    rb = pgen.tile([128, QB], F32, tag="mm")
                    for k in range(2):          # heads within the pair
                        rp = k * HS
                        nc.tensor.matmul(rb[rp:rp + HS, :],
                                         lhsT=ones[rp:rp + 1, :],
                                         rhs=r16[rp:rp + 1, :],
                                         start=True, stop=True)
                    nc.vector.tensor_tensor(out=yTu[:, hp, cols],
                                            in0=yTu[:, hp, cols],
                                            in1=rb, op=ALU.mult)

                return COPIES, NORM

            # ---------------- pipelined emission ----------------
            # Filler budget is balanced against each q-block's exp load
            # (slots = 2*(qj+1) per pair): block n+1's qk/v-gen runs during
            # attention of block n, except half of block 3's which shifts
            # into block 3 itself (its VMMs first: AV of qj=3 reads v t12-15
            # from step 6, so they must emit before those AVs hit the PE
            # queue).  proj(n) runs during attention of block n+1.
            def tq(n, ms, ts=()):
                return ([make_qk_closure(n, m) for m in ms]
                        + [make_vmm_closure(t) for t in ts])

            inj_plan = {
                0: tq(0, [2, 6, 3, 7]) + tq(1, [0, 4, 1, 5, 2, 6, 3, 7], range(4, 8)),
                1: tq(2, [0, 4, 1, 5, 2, 6, 3, 7], range(8, 12)),
                2: tq(3, [0, 4, 1, 5, 2, 6, 3, 7], range(12, 16)),
                3: [],
            }
            for cl in tq(0, [0, 4, 1, 5], range(0, 4)):
                cl()
            carry = []      # previous pair's COPIES: step 0 of the next pair
            norm = None     # previous pair's NORM: first filler (step 1)
            for qj in range(NQ):
                inj = list(inj_plan[qj])
                if qj == 3:
                    # proj runs in the exp-heaviest phase to keep the PE fed
                    inj += (make_proj_closures(0) + make_proj_closures(1)
                            + make_proj_closures(2))
                for hp in range(4):
                    # hand this pair its even share of the filler closures
                    per = (len(inj) + (4 - hp) - 1) // (4 - hp)
                    filler = ([norm] if norm else []) + inj[:per]
                    del inj[:per]
                    pyA, pyB = emit_pair(qj, hp, carry, filler)
                    cp, norm = make_finish_closures(qj, hp, pyA, pyB)
                    carry = [cp]
                assert not inj
            for cl in carry:
                cl()
            norm()
            for cl in make_proj_closures(NQ - 1):
                cl()

    nc.compile()
    return nc


def make_masks():
    kk = np.arange(128)[:, None]
    qq = np.arange(QB)[None, :]
    m = np.zeros((4, 128, QB), dtype=ml_dtypes.bfloat16)
    for d in range(4):
        m[d] = (qq >= kk + d * 128).astype(ml_dtypes.bfloat16)
    return m


def make_in_maps(x, w_attn, b_attn, w_proj, b_proj):
    masks = make_masks()
    in_maps = []
    for core in range(N_CORES):
        b, g = core // 2, core % 2
        cq = slice(g * GC, (g + 1) * GC)
        ck = slice(C + g * GC, C + (g + 1) * GC)
        cv = slice(2 * C + g * GC, 2 * C + (g + 1) * GC)
        w_qkv_g = np.concatenate([w_attn[:, cq], w_attn[:, ck], w_attn[:, cv]], axis=1)
        # re-tile per 128-col m-block: [NM, 128 partitions, CK*128] so each
        # m-block DMA reads contiguous 2KB per partition
        # host layout == SBUF layout [128, CK, NM*128], m-blocks in the
        # DMA-needed order so the 5 w triggers are plain contiguous slices
        w_order = [0, 4, 8, 9, 10, 11, 1, 5, 2, 6, 3, 7]
        w_tiled = np.ascontiguousarray(
            w_qkv_g.astype(ml_dtypes.bfloat16)
            .reshape(CK, 128, NM, 128)[:, :, w_order]
            .transpose(1, 0, 2, 3).reshape(128, CK, NM * 128))
        in_maps.append({
            "xT": np.ascontiguousarray(np.asarray(x[b], dtype=np.float32).T
                                       .astype(ml_dtypes.bfloat16)),
            "w_qkv": w_tiled,
            # pre-tiled [128, 8]: b_qk[p, m] = flat[m*128 + p] (contiguous DMA)
            "b_qk": np.ascontiguousarray(
                np.concatenate([b_attn[cq], b_attn[ck]]).astype(np.float32)
                .reshape(8, 128).T),
            "b_v": np.ascontiguousarray(b_attn[cv]).astype(np.float32),
            "w_proj": np.ascontiguousarray(w_proj[g * GC:(g + 1) * GC, :].astype(ml_dtypes.bfloat16)),
            "b_proj": (b_proj if g == 0 else np.zeros_like(b_proj)).astype(np.float32),
            "masks": masks,
        })
    return in_maps


_PROGRAM = None


def kernel(x, w_attn, b_attn, w_proj, b_proj, _trace=False):
    global _PROGRAM
    x = np.asarray(x)
    B = x.shape[0]
    if _PROGRAM is None:
        _PROGRAM = build_program()
    nc = _PROGRAM
    in_maps = make_in_maps(x, np.asarray(w_attn), np.asarray(b_attn),
                           np.asarray(w_proj), np.asarray(b_proj))
    res = bass_utils.run_bass_kernel_spmd(nc, in_maps, core_ids=list(range(N_CORES)),
                                          trace=_trace)
    y = np.zeros((B, T, C), np.float32)
    for b in range(B):
        y[b] = (res.results[2 * b]["y"].astype(np.float32)
                + res.results[2 * b + 1]["y"].astype(np.float32))
    if _trace:
        return y, res
    return y
